# revision 2
# baseline (speedup 1.0000x reference)
"""Trainium2 Bass kernel for nn_MoELayer (top-1 MoE, dense-masked reference).

Strategy
--------
The reference runs every expert's MLP over every token and then keeps only
the output of each token's argmax-gated expert.  Mathematically the output
for token t is exactly `mlp_{top1(t)}(x_t)`, so we:

  1. compute the (tiny) gate + argmax on the host in float64,
  2. group tokens by chosen expert (expert-parallel sharding: core e gets
     expert e's weights and the tokens routed to it, padded to a fixed
     capacity C),
  3. run one dense MLP per core on its token batch:
        yT = W2^T @ relu(W1^T @ xT + b1) + b2
  4. scatter the per-expert outputs back into the full [B,T,D] tensor.

This does 1/E of the reference FLOPs.  All matmul operands are bf16
(fp32 PSUM accumulation): same 1-cycle/row PE rate as fp32r but half the
HBM traffic and SBUF footprint, which lets the entire hidden activation
h [H, C] stay resident in SBUF.  Phase B then contracts all of H in one
32-matmul PSUM accumulation chain per output tile -- no vector-engine
read-modify-write accumulation, maximally long gapless PE chains.

Weights/x are prepacked on the host into the exact SBUF layout so every
DMA is a single large contiguous-per-partition transfer (1-2 MB).
bf16 rounding of x/W1/W2/h gives ~3e-3 relative error, well inside the
2e-2 tolerance.
"""

import os
import sys

import numpy as np

for _p in ("/opt/trn_rl_repo", "/root/.axon_site/_ro/trn_rl_repo"):
    if os.path.isdir(_p) and _p not in sys.path:
        sys.path.insert(0, _p)

import concourse.bass as bass
import concourse.bacc as bacc
import concourse.mybir as mybir
from concourse.bass_utils import run_bass_kernel_spmd
from concourse.tile import TileContext

import ml_dtypes

# run_bass_kernel_spmd's trace path (BASS_TRACE=1) imports antenv.axon_hooks,
# which not every container ships; force tracing off when it's absent so a
# stray env var can't crash the run.
try:
    from antenv.axon_hooks import get_axon_ntff_profile_hook  # noqa: F401
except Exception:
    os.environ["BASS_NEVER_TRACE"] = "1"

B, T, D, H, E = 4, 2048, 1024, 4096, 8
BT = B * T
N_CORES = 8
F32 = mybir.dt.float32
BF16 = mybir.dt.bfloat16
BF16NP = ml_dtypes.bfloat16
AF = mybir.ActivationFunctionType

ND = D // 128    # 8   d-axis 128-row tiles
NHT = H // 128   # 32  h-axis 128-row tiles
NW1C = 8         # w1 streamed in 8 chunks of NHT/NW1C h-tiles
GPC = NHT // NW1C  # 4 h-tiles per w1 chunk

# SBUF budget (bytes/partition): ht 64*C + xt 16*C + w2 64K + w1 2*8K +
# yb 4*2K + consts.  C=1280 -> ~190KB, the usable limit.
C_CAP = 1280

_PROGRAM_CACHE: dict[int, bass.Bass] = {}
LAST_RESULT = None  # BassKernelResults of the most recent device run (for test.py)


def _token_tiles(C):
    """Split C tokens into matmul moving-dim tiles of up to 512 columns
    (512 fp32 = one PSUM bank). bf16 operands keep full PE rate at any
    moving size, so the remainder tile can be small."""
    assert C % 8 == 0
    tiles = []
    t0 = 0
    while C - t0 >= 512:
        tiles.append((t0, 512))
        t0 += 512
    if C - t0:
        tiles.append((t0, C - t0))
    return tiles


def _build_program(C: int, repeats: int = 1) -> bass.Bass:
    """One expert MLP over C tokens: yT[D,C] = W2^T @ relu(W1^T @ xT + b1) + b2.

    All 8 cores run this same program on different data (SPMD).

    `repeats` re-runs the whole (idempotent) compute body that many times
    inside one NEFF — used only by test.py to amplify kernel time above the
    axon per-execution launch overhead when measuring.
    """
    nc = bacc.Bacc("TRN2", target_bir_lowering=False, debug=False)

    xb = nc.dram_tensor("xb", [128, ND * C], BF16, kind="ExternalInput").ap()
    w1p = nc.dram_tensor("w1p", [128, NHT * ND * 128], BF16, kind="ExternalInput").ap()
    w2p = nc.dram_tensor("w2p", [128, NHT * D], BF16, kind="ExternalInput").ap()
    b1c = nc.dram_tensor("b1c", [128, NHT], F32, kind="ExternalInput").ap()
    b2c = nc.dram_tensor("b2c", [128, ND], F32, kind="ExternalInput").ap()
    yT = nc.dram_tensor("yT", [D, C], F32, kind="ExternalOutput").ap()

    W1CW = GPC * ND * 128  # 4096 cols per streamed w1 chunk
    ttiles = _token_tiles(C)

    with TileContext(nc) as tc:
        with (
            tc.tile_pool(name="const", bufs=1) as constp,
            tc.tile_pool(name="xp", bufs=1) as xp,
            tc.tile_pool(name="htp", bufs=1) as htp,
            tc.tile_pool(name="w2pool", bufs=1) as w2pool,
            tc.tile_pool(name="w1pool", bufs=2) as w1pool,
            tc.tile_pool(name="ybp", bufs=4) as ybp,
            tc.tile_pool(name="psA", bufs=4, space="PSUM") as psA,
            tc.tile_pool(name="psB", bufs=4, space="PSUM") as psB,
        ):
            # Warmup ACT with no cross-engine waits: walrus's lower_act
            # attaches the one-time activation-table load to the first ACT
            # instruction, consuming one of its two sync-wait slots. Give it
            # a dependency-free instruction so real ACTs keep both slots.
            warm = constp.tile([128, 1], F32, tag="warm")
            nc.scalar.memzero(warm[:, :])
            nc.scalar.activation(warm[:, :], warm[:, :], AF.Relu)
            nc.scalar.activation(warm[:, :], warm[:, :], AF.Identity)

            b1t = constp.tile([128, NHT], F32, tag="b1t")
            b2t = constp.tile([128, ND], F32, tag="b2t")

            for rep in range(repeats):
                # x resident: partition = d within 128-chunk, col block dc
                # holds xT[dc*128:(dc+1)*128, :]. Single 2MB-class DMA.
                xt = xp.tile([128, ND * C], BF16, tag="xt")
                nc.sync.dma_start(xt[:, :], xb)
                if rep == 0:
                    nc.sync.dma_start(b1t[:, :], b1c)
                    nc.sync.dma_start(b2t[:, :], b2c)

                # W2 resident for the whole of phase B; prefetch on the ACT
                # HWDGE ring so it never queues behind the w1/x stream on
                # the SP ring. Lands well before phase A (~117us) ends.
                w2t = w2pool.tile([128, NHT * D], BF16, tag="w2t")
                for q in range(4):
                    qs = (NHT * D // 4) * q
                    nc.scalar.dma_start(
                        w2t[:, qs : qs + NHT * D // 4], w2p[:, qs : qs + NHT * D // 4]
                    )

                # full hidden activation resident: col block g holds
                # relu(W1^T x + b1)[g*128:(g+1)*128, :] in bf16
                ht = htp.tile([128, NHT * C], BF16, tag="ht")

                # ---- Phase A: ht = relu(W1^T @ x + b1) ----
                for wc in range(NW1C):
                    w1t = w1pool.tile([128, W1CW], BF16, tag="w1c")
                    nc.sync.dma_start(w1t[:, :], w1p[:, wc * W1CW : (wc + 1) * W1CW])
                    for gi in range(GPC):
                        g = wc * GPC + gi
                        for t0, tn in ttiles:
                            ps = psA.tile([128, 512], F32, tag="psA")
                            for dc in range(ND):
                                nc.tensor.matmul(
                                    ps[:, :tn],
                                    w1t[:, (gi * ND + dc) * 128 : (gi * ND + dc + 1) * 128],
                                    xt[:, dc * C + t0 : dc * C + t0 + tn],
                                    start=(dc == 0),
                                    stop=(dc == ND - 1),
                                )
                            nc.scalar.activation(
                                ht[:, g * C + t0 : g * C + t0 + tn],
                                ps[:, :tn],
                                AF.Relu,
                                bias=b1t[:, g : g + 1],
                            )

                # ---- Phase B: yT = W2^T @ ht + b2, one 32-matmul PSUM
                # accumulation chain per [128, tn] output tile ----
                for dt in range(ND):
                    for t0, tn in ttiles:
                        ps = psB.tile([128, 512], F32, tag="psB")
                        for hs in range(NHT):
                            nc.tensor.matmul(
                                ps[:, :tn],
                                w2t[:, hs * D + dt * 128 : hs * D + dt * 128 + 128],
                                ht[:, hs * C + t0 : hs * C + t0 + tn],
                                start=(hs == 0),
                                stop=(hs == NHT - 1),
                            )
                        yb = ybp.tile([128, 512], F32, tag="yb")
                        nc.scalar.activation(
                            yb[:, :tn], ps[:, :tn], AF.Identity, bias=b2t[:, dt : dt + 1]
                        )
                        nc.scalar.dma_start(
                            yT[dt * 128 : (dt + 1) * 128, t0 : t0 + tn], yb[:, :tn]
                        )

    nc.compile()
    return nc


def _get_program(C: int) -> bass.Bass:
    if C not in _PROGRAM_CACHE:
        _PROGRAM_CACHE[C] = _build_program(C)
    return _PROGRAM_CACHE[C]


def _pack_x(xe_bf: np.ndarray, C: int) -> np.ndarray:
    """[C, D] bf16 -> [128, ND*C] with col block dc = xT[dc*128:(dc+1)*128, :]."""
    xeT = np.ascontiguousarray(xe_bf.T)  # [D, C]
    return np.ascontiguousarray(
        xeT.reshape(ND, 128, C).transpose(1, 0, 2)
    ).reshape(128, ND * C)


def _prepare(x, Wg, bg, W1, b1, W2, b2):
    """Host routing: fp64 gate + argmax, group tokens by expert, build the
    per-core (per-expert) input maps padded to capacity C."""
    xf = np.ascontiguousarray(np.asarray(x, dtype=np.float32).reshape(BT, D))

    # Host gate in float64: scores are tiny (BT x E) and fp64 argmax is
    # robust to any fp32 accumulation-order noise in the reference.
    scores = xf.astype(np.float64) @ np.asarray(Wg, dtype=np.float64)
    scores += np.asarray(bg, dtype=np.float64)
    top1 = np.argmax(scores, axis=-1)

    counts = np.bincount(top1, minlength=E)
    # 8-aligned capacity (32B DMA lines); above C_CAP the resident x/h
    # tiles exceed the SBUF budget, so larger routing skews fall back to
    # multiple passes (never hit for the ~1.1k-per-expert counts this
    # gate produces).
    C = max(512, int(np.ceil(counts.max() / 8)) * 8)
    C = min(C, C_CAP)

    xbf = xf.astype(BF16NP)
    W1f = np.asarray(W1, dtype=np.float32)
    b1f = np.asarray(b1, dtype=np.float32)
    W2f = np.asarray(W2, dtype=np.float32)
    b2f = np.asarray(b2, dtype=np.float32)

    in_maps = []
    idxs = []
    for e in range(E):
        idx = np.nonzero(top1 == e)[0]
        idxs.append(idx)
        xe = np.zeros((C, D), dtype=BF16NP)
        xe[: min(len(idx), C)] = xbf[idx[:C]]
        w1bf = W1f[e].astype(BF16NP)  # [D, H]
        w2bf = W2f[e].astype(BF16NP)  # [H, D]
        in_maps.append(
            {
                "xb": _pack_x(xe, C),
                # w1p[p, g*1024 + dc*128 + j] = W1[dc*128+p, g*128+j]
                "w1p": np.ascontiguousarray(
                    w1bf.reshape(ND, 128, NHT, 128).transpose(1, 2, 0, 3)
                ).reshape(128, NHT * ND * 128),
                # w2p[p, hs*1024 + j] = W2[hs*128+p, j]
                "w2p": np.ascontiguousarray(
                    w2bf.reshape(NHT, 128, D).transpose(1, 0, 2)
                ).reshape(128, NHT * D),
                "b1c": np.ascontiguousarray(b1f[e].reshape(NHT, 128).T),
                "b2c": np.ascontiguousarray(b2f[e].reshape(ND, 128).T),
            }
        )
    return C, in_maps, idxs


_FASTPATH_CACHE: dict[int, object] = {}


def _make_fastpath(nc):
    """Memoized version of run_bass_kernel_spmd's axon execution path: the
    same sharded custom-call jit, kept alive so repeat kernel() calls skip
    jax retracing and NEFF reload. Numerically identical machinery."""
    import jax
    from jax.sharding import Mesh, PartitionSpec
    from jax.experimental.shard_map import shard_map
    from concourse.bass2jax import (
        _bass_exec_p,
        install_neuronx_cc_hook,
        partition_id_tensor,
    )

    install_neuronx_cc_hook()
    partition_name = nc.partition_id_tensor.name if nc.partition_id_tensor else None
    in_names, out_names, out_avals = [], [], []
    for alloc in nc.m.functions[0].allocations:
        if not isinstance(alloc, mybir.MemoryLocationSet):
            continue
        name = alloc.memorylocations[0].name
        if alloc.kind == "ExternalInput":
            if name != partition_name:
                in_names.append(name)
        elif alloc.kind == "ExternalOutput":
            out_names.append(name)
            out_avals.append(
                jax.core.ShapedArray(tuple(alloc.tensor_shape), mybir.dt.np(alloc.dtype))
            )
    all_names = in_names + out_names + ([partition_name] if partition_name else [])

    def _body(*args):
        operands = list(args)
        if partition_name is not None:
            operands.append(partition_id_tensor())
        return tuple(
            _bass_exec_p.bind(
                *operands,
                out_avals=tuple(out_avals),
                in_names=tuple(all_names),
                out_names=tuple(out_names),
                lowering_input_output_aliases=(),
                sim_require_finite=True,
                sim_require_nnan=True,
                nc=nc,
            )
        )

    mesh = Mesh(np.asarray(jax.devices()[:N_CORES]), ("core",))
    nin, nout = len(in_names), len(out_names)
    fn = jax.jit(
        shard_map(
            _body,
            mesh=mesh,
            in_specs=(PartitionSpec("core"),) * (nin + nout),
            out_specs=(PartitionSpec("core"),) * nout,
            check_rep=False,
        )
    )

    def run(in_maps):
        args = [
            np.concatenate([np.asarray(m[nm]) for m in in_maps], axis=0)
            for nm in in_names
        ]
        for aval in out_avals:
            args.append(np.zeros((N_CORES * aval.shape[0], *aval.shape[1:]), aval.dtype))
        outs = fn(*args)
        return [
            {
                nm: np.asarray(outs[i]).reshape(N_CORES, *out_avals[i].shape)[c]
                for i, nm in enumerate(out_names)
            }
            for c in range(N_CORES)
        ]

    return run


def _run_spmd(C, nc, in_maps):
    global LAST_RESULT
    if C in _FASTPATH_CACHE:
        return _FASTPATH_CACHE[C](in_maps)
    # First call per capacity: the prescribed run_bass_kernel_spmd path
    # (compiles the NEFF); then build the memoized executable for repeats.
    res = run_bass_kernel_spmd(nc, in_maps, list(range(N_CORES)))
    LAST_RESULT = res
    try:
        _FASTPATH_CACHE[C] = _make_fastpath(nc)
    except Exception:
        pass
    return res.results


def kernel(x, Wg, bg, W1, b1, W2, b2):
    C, in_maps, idxs = _prepare(x, Wg, bg, W1, b1, W2, b2)
    nc = _get_program(C)
    results = _run_spmd(C, nc, in_maps)

    out = np.empty((BT, D), dtype=np.float32)
    for e in range(E):
        n_e = min(len(idxs[e]), C)
        if n_e:
            out[idxs[e][:n_e]] = results[e]["yT"][:, :n_e].T

    # Overflow passes: only if some expert drew more than C (=C_CAP) tokens,
    # which this gate's near-uniform routing never does for the given data.
    max_count = max(len(i) for i in idxs)
    done = C
    while done < max_count:
        xf = np.asarray(x, dtype=np.float32).reshape(BT, D).astype(BF16NP)
        for e in range(E):
            idx = idxs[e][done : done + C]
            xe = np.zeros((C, D), dtype=BF16NP)
            xe[: len(idx)] = xf[idx]
            in_maps[e]["xb"] = _pack_x(xe, C)
        results = _run_spmd(C, nc, in_maps)
        for e in range(E):
            idx = idxs[e][done : done + C]
            if len(idx):
                out[idx] = results[e]["yT"][:, : len(idx)].T
        done += C

    return out.reshape(B, T, D)
